# revision 1
# baseline (speedup 1.0000x reference)
"""MentionScore kernel for 8 Trainium2 NeuronCores.

Strategy (data-parallel over spans, hardcoded shapes):
  T=8192 tokens, A=1024, E=512, HID=150, L=10, S=32768 spans, 8 cores.
  Key algebraic rewrite: with g = [states[st], states[en], pooled, size_emb],
    h1 = g @ Ws1 = states[st] @ W_st + states[en] @ W_en
         + sum_l w[s,l] * (embeds[idx] @ W_em) + width_table[bucket] @ W_wd
  so all big matmuls act on per-token tables, and the ragged gathers become
  tiny banded matmuls with 0/1 (or softmax-weight) band matrices.

  Spans are sorted by start, so 128 consecutive spans touch a ~42-token
  window.  Each 128-span tile gets a 64-token window; two tiles are packed
  into one 128-partition "pair tile".  Per pair tile the kernel computes the
  attention MLP and the projected tables for those 128 window tokens, then
  per half contracts band matrices (built host-side from the integer span
  indices) against the tables.  The softmax over each span is realized as
  exp(attn) * mask followed by a matmul against a ones column (denominator)
  and a per-span reciprocal scale.

All eight cores run the same program on their own 4096-span slice; the only
host-side float math is folding weights/biases (parameter preprocessing).
Inputs are packed into few DRAM tensors so each matmul waits on few sems.
"""

import numpy as np
import os
import sys

sys.path.insert(0, "/opt/trn_rl_repo")

T, A, E, D, S = 8192, 1024, 512, 20, 32768
HID, L = 150, 10
NCORES = 8
SPC = S // NCORES            # spans per core = 4096
NTILE = SPC // 128           # span tiles per core = 32
NPAIR = NTILE // 2           # pair tiles per core = 16
BINS = np.array([1, 2, 3, 4, 8, 16, 32, 64], dtype=np.int64)
KS = 9                       # K chunks of 128 for states (1024 + bias row)
KE = 4                       # K chunks of 128 for embeds
WIN = 64

# column offsets inside the packed const tensor [128, CW]
_off = 0
def _seg(n):
    global _off
    o = _off
    _off += n
    return o
O_WA1 = _seg(KS * HID)
O_WSE = _seg(KS * 2 * HID)
O_WEM = _seg(KE * HID)
O_WA2LO = _seg(HID)
O_WA2HI = _seg(HID)   # rows 0:22
O_WA2B = _seg(HID)    # row 0
O_WS2LO = _seg(HID)
O_WS2HI = _seg(HID)   # rows 0:22
O_WS2B = _seg(HID)    # row 0
O_WA3 = _seg(HID)     # broadcast to 128 rows
O_WS3 = _seg(HID)
O_TWID = _seg(HID)    # rows 0:9
O_SCAL = _seg(2)      # col0=ba3 col1=bs3 (all rows)
O_ID = _seg(128)
CW = _off


_PROG_CACHE = {}


def _build_program(win):
    assert win == WIN
    if win in _PROG_CACHE:
        return _PROG_CACHE[win]
    import concourse.bass as bass
    import concourse.mybir as mybir
    from concourse import tile
    from concourse.bacc import Bacc

    f32 = mybir.dt.float32
    bf16 = mybir.dt.bfloat16
    AF = mybir.ActivationFunctionType
    ALU = mybir.AluOpType
    AX = mybir.AxisListType

    nc = Bacc()

    d_big = nc.dram_tensor("big", [NPAIR, 128, (KS + KE) * 128], f32,
                           kind="ExternalInput")
    d_bm = nc.dram_tensor("bm", [NPAIR, 128, 3 * 256], bf16, kind="ExternalInput")
    d_bkt = nc.dram_tensor("bkt", [NPAIR, 9, 256], f32, kind="ExternalInput")
    d_wp = nc.dram_tensor("wpack", [128, CW], f32, kind="ExternalInput")
    d_out = nc.dram_tensor("scores", [NTILE, 128, 1], f32, kind="ExternalOutput")

    with tile.TileContext(nc) as tc:
        with (
            tc.tile_pool(name="const", bufs=1) as cpool,
            tc.tile_pool(name="io", bufs=3) as iop,
            tc.tile_pool(name="work", bufs=3) as wp,
            tc.tile_pool(name="psM", bufs=2, space=bass.MemorySpace.PSUM) as psM,
            tc.tile_pool(name="psT", bufs=1, space=bass.MemorySpace.PSUM) as psT,
            tc.tile_pool(name="psB", bufs=1, space=bass.MemorySpace.PSUM) as psB,
        ):
            W = cpool.tile([128, CW], f32)
            for k in range(KS):
                nc.sync.dma_start(W[:, O_WA1 + HID * k:O_WA1 + HID * (k + 1)],
                                  d_wp[:, O_WA1 + HID * k:O_WA1 + HID * (k + 1)])
                nc.sync.dma_start(
                    W[:, O_WSE + 2 * HID * k:O_WSE + 2 * HID * (k + 1)],
                    d_wp[:, O_WSE + 2 * HID * k:O_WSE + 2 * HID * (k + 1)])
            for k in range(KE):
                nc.sync.dma_start(W[:, O_WEM + HID * k:O_WEM + HID * (k + 1)],
                                  d_wp[:, O_WEM + HID * k:O_WEM + HID * (k + 1)])
            nc.sync.dma_start(W[:, O_WA2LO:O_ID + 128], d_wp[:, O_WA2LO:O_ID + 128])
            zbias = cpool.tile([128, 1], f32)
            nc.gpsimd.memset(zbias[:], 0.0)
            ones1 = cpool.tile([1, 128], f32)
            nc.gpsimd.memset(ones1[:], 1.0)
            ident = W[:, O_ID:O_ID + 128]

            # PE observes each W-segment DMA queue once, so later matmuls
            # re-reading W never need a second wait slot.
            dum = psB.tile([1, 1], f32, tag="dummy")
            segs = ([O_WA1 + HID * k for k in range(KS)]
                    + [O_WSE + 2 * HID * k for k in range(KS)]
                    + [O_WEM + HID * k for k in range(KE)]
                    + [O_WA2LO])
            for c0 in segs:
                nc.tensor.matmul(dum[:], W[0:1, c0:c0 + 1], W[0:1, c0:c0 + 1],
                                 start=True, stop=True)

            for p in range(NPAIR):
                big = iop.tile([128, (KS + KE) * 128], f32, tag="big")
                for k in range(KS + KE):
                    nc.sync.dma_start(big[:, 128 * k:128 * (k + 1)],
                                      d_big[p, :, 128 * k:128 * (k + 1)])
                bm = iop.tile([128, 3 * 256], bf16, tag="bm")
                for k in range(3):
                    nc.sync.dma_start(bm[:, 256 * k:256 * (k + 1)],
                                      d_bm[p, :, 256 * k:256 * (k + 1)])
                bkt = iop.tile([9, 256], f32, tag="bkt")
                nc.sync.dma_start(bkt[:], d_bkt[p])

                # ---- attention MLP over the 128 window tokens ----
                a1p = psM.tile([128, HID], f32, tag="mm150")
                for k in range(KS):
                    nc.tensor.matmul(a1p[:], big[:, 128 * k:128 * (k + 1)],
                                     W[:, O_WA1 + HID * k:O_WA1 + HID * (k + 1)],
                                     start=(k == 0), stop=(k == KS - 1))
                a1r = wp.tile([128, HID], f32, tag="a1r")
                nc.scalar.activation(a1r[:], a1p[:], AF.Relu, bias=zbias[:])

                tp1 = psB.tile([128, 128], f32, tag="tpA")
                nc.tensor.transpose(tp1[:], a1r[:, 0:128], ident)
                tp2 = psB.tile([22, 128], f32, tag="tpB")
                nc.tensor.transpose(tp2[:], a1r[:, 128:HID], ident)
                a1Tlo = wp.tile([128, 128], f32, tag="a1Tlo")
                nc.vector.tensor_copy(a1Tlo[:], tp1[:])
                a1Thi = wp.tile([22, 128], f32, tag="a1Thi")
                nc.vector.tensor_copy(a1Thi[:], tp2[:])

                a2p = psM.tile([128, HID], f32, tag="mm150")
                nc.tensor.matmul(a2p[:], a1Tlo[:], W[:, O_WA2LO:O_WA2LO + HID],
                                 start=True, stop=False)
                nc.tensor.matmul(a2p[:], a1Thi[:], W[0:22, O_WA2HI:O_WA2HI + HID],
                                 start=False, stop=False)
                nc.tensor.matmul(a2p[:], ones1[:], W[0:1, O_WA2B:O_WA2B + HID],
                                 start=False, stop=True)
                a2r = wp.tile([128, HID], f32, tag="a2r")
                nc.scalar.activation(a2r[:], a2p[:], AF.Relu, bias=zbias[:])

                atmp = wp.tile([128, HID], f32, tag="atmp")
                nc.vector.tensor_tensor(atmp[:], a2r[:], W[:, O_WA3:O_WA3 + HID],
                                        op=ALU.mult)
                adot = wp.tile([128, 1], f32, tag="adot")
                nc.vector.tensor_reduce(adot[:], atmp[:], axis=AX.X, op=ALU.add)
                ew = wp.tile([128, 1], f32, tag="ew")
                nc.scalar.activation(ew[:], adot[:], AF.Exp,
                                     bias=W[:, O_SCAL:O_SCAL + 1])

                # ---- projected tables for the 128 window tokens ----
                tables = psT.tile([128, 3 * HID], f32, tag="tables")
                tsep = tables[:, 0:2 * HID]
                for k in range(KS):
                    nc.tensor.matmul(tsep[:, :], big[:, 128 * k:128 * (k + 1)],
                                     W[:, O_WSE + 2 * HID * k:O_WSE + 2 * HID * (k + 1)],
                                     start=(k == 0), stop=(k == KS - 1))
                tse = wp.tile([128, 2 * HID], f32, tag="tse")
                nc.vector.tensor_copy(tse[:], tsep[:, :])

                tembp = tables[:, 2 * HID:3 * HID]
                for k in range(KE):
                    nc.tensor.matmul(tembp[:, :], big[:, 128 * (KS + k):128 * (KS + k + 1)],
                                     W[:, O_WEM + HID * k:O_WEM + HID * (k + 1)],
                                     start=(k == 0), stop=(k == KE - 1))
                temb = wp.tile([128, HID + 1], f32, tag="temb")
                nc.vector.tensor_copy(temb[:, 0:HID], tembp[:, :])
                nc.vector.memset(temb[:, HID:HID + 1], 1.0)

                # softmax numerators in band layout: exp(attn[c]) * mask[c,s]
                bw = wp.tile([128, 256], f32, tag="bw")
                nc.vector.tensor_scalar_mul(bw[:], bm[:, 512:768], ew[:])
                bstf = wp.tile([128, 256], f32, tag="bstf")
                nc.vector.tensor_copy(bstf[:], bm[:, 0:256])
                benf = wp.tile([128, 256], f32, tag="benf")
                nc.vector.tensor_copy(benf[:], bm[:, 256:512])

                for h in range(2):
                    o = win * h
                    sc = slice(128 * h, 128 * h + 128)
                    hA = psB.tile([128, HID], f32, tag="hA")
                    nc.tensor.matmul(hA[:], bstf[o:o + win, sc],
                                     tse[o:o + win, 0:HID], start=True, stop=False)
                    nc.tensor.matmul(hA[:], benf[o:o + win, sc],
                                     tse[o:o + win, HID:2 * HID],
                                     start=False, stop=False)
                    nc.tensor.matmul(hA[:], bkt[:, sc], W[0:9, O_TWID:O_TWID + HID],
                                     start=False, stop=True)
                    hB = psB.tile([128, HID + 1], f32, tag="hB")
                    nc.tensor.matmul(hB[:], bw[o:o + win, sc], temb[o:o + win, :],
                                     start=True, stop=True)

                    rec = wp.tile([128, 1], f32, tag="rec")
                    nc.vector.reciprocal(rec[:], hB[:, HID:HID + 1])
                    hBs = wp.tile([128, HID], f32, tag="hBs")
                    nc.vector.tensor_scalar_mul(hBs[:], hB[:, 0:HID], rec[:])
                    h1s = wp.tile([128, HID], f32, tag="h1s")
                    nc.vector.tensor_tensor(h1s[:], hA[:], hBs[:], op=ALU.add)
                    h1r = wp.tile([128, HID], f32, tag="h1r")
                    nc.scalar.activation(h1r[:], h1s[:], AF.Relu, bias=zbias[:])

                    tq1 = psB.tile([128, 128], f32, tag="tpA")
                    nc.tensor.transpose(tq1[:], h1r[:, 0:128], ident)
                    tq2 = psB.tile([22, 128], f32, tag="tpB")
                    nc.tensor.transpose(tq2[:], h1r[:, 128:HID], ident)
                    h1Tlo = wp.tile([128, 128], f32, tag="h1Tlo")
                    nc.vector.tensor_copy(h1Tlo[:], tq1[:])
                    h1Thi = wp.tile([22, 128], f32, tag="h1Thi")
                    nc.vector.tensor_copy(h1Thi[:], tq2[:])

                    h2p = psM.tile([128, HID], f32, tag="mm150")
                    nc.tensor.matmul(h2p[:], h1Tlo[:], W[:, O_WS2LO:O_WS2LO + HID],
                                     start=True, stop=False)
                    nc.tensor.matmul(h2p[:], h1Thi[:], W[0:22, O_WS2HI:O_WS2HI + HID],
                                     start=False, stop=False)
                    nc.tensor.matmul(h2p[:], ones1[:], W[0:1, O_WS2B:O_WS2B + HID],
                                     start=False, stop=True)
                    h2r = wp.tile([128, HID], f32, tag="h2r")
                    nc.scalar.activation(h2r[:], h2p[:], AF.Relu, bias=zbias[:])

                    stmp = wp.tile([128, HID], f32, tag="stmp")
                    nc.vector.tensor_tensor(stmp[:], h2r[:], W[:, O_WS3:O_WS3 + HID],
                                            op=ALU.mult)
                    sdot = wp.tile([128, 1], f32, tag="sdot")
                    nc.vector.tensor_reduce(sdot[:], stmp[:], axis=AX.X, op=ALU.add)
                    sout = wp.tile([128, 1], f32, tag="sout")
                    nc.vector.tensor_tensor(sout[:], sdot[:],
                                            W[:, O_SCAL + 1:O_SCAL + 2], op=ALU.add)
                    nc.sync.dma_start(d_out[2 * p + h], sout[:])

    return nc


def _prep_inputs(states, embeds, starts, lengths,
                 Wa1, ba1, Wa2, ba2, Wa3, ba3,
                 width_table, Ws1, bs1, Ws2, bs2, Ws3, bs3, win):
    f32 = np.float32
    import ml_dtypes
    bf16 = ml_dtypes.bfloat16

    ends = starts + lengths
    bucket = np.searchsorted(BINS, lengths + 1, side="right")

    # augmented transposed states: rows 0:1024 states.T, row 1024 ones, pad
    sTa = np.zeros((KS * 128, T), dtype=f32)
    sTa[0:A] = np.asarray(states, dtype=f32).T
    sTa[A] = 1.0
    eTa = np.asarray(embeds, dtype=f32).T.copy()      # [512, T]

    t0 = starts[0::128].copy()                        # [256] span-tile windows
    t0c = t0.reshape(NCORES, NTILE)
    col_idx = np.minimum(
        t0c[:, :, None] + np.arange(win)[None, None, :], T - 1
    ).reshape(NCORES, NPAIR, 2 * win)

    Ws1 = np.asarray(Ws1, f32)
    wpack = np.zeros((128, CW), dtype=f32)

    def kchunk_into(off, w, kc, ncol):
        w3 = w.reshape(kc, 128, ncol)
        for k in range(kc):
            wpack[:, off + ncol * k:off + ncol * (k + 1)] = w3[k]

    Wa1a = np.zeros((KS * 128, HID), dtype=f32)
    Wa1a[0:A] = np.asarray(Wa1, f32)
    Wa1a[A] = np.asarray(ba1, f32)
    kchunk_into(O_WA1, Wa1a, KS, HID)

    Wse = np.zeros((KS * 128, 2 * HID), dtype=f32)
    Wse[0:A, 0:HID] = Ws1[0:A]
    Wse[A, 0:HID] = np.asarray(bs1, f32)
    Wse[0:A, HID:] = Ws1[A:2 * A]
    kchunk_into(O_WSE, Wse, KS, 2 * HID)

    Wem = np.zeros((KE * 128, HID), dtype=f32)
    Wem[0:E] = Ws1[2 * A:2 * A + E]
    kchunk_into(O_WEM, Wem, KE, HID)

    Wa2a = np.zeros((151, HID), dtype=f32)
    Wa2a[0:HID] = np.asarray(Wa2, f32)
    Wa2a[HID] = np.asarray(ba2, f32)
    wpack[:, O_WA2LO:O_WA2LO + HID] = Wa2a[0:128]
    wpack[0:22, O_WA2HI:O_WA2HI + HID] = Wa2a[128:150]
    wpack[0:1, O_WA2B:O_WA2B + HID] = Wa2a[150:151]

    Ws2a = np.zeros((151, HID), dtype=f32)
    Ws2a[0:HID] = np.asarray(Ws2, f32)
    Ws2a[HID] = np.asarray(bs2, f32)
    wpack[:, O_WS2LO:O_WS2LO + HID] = Ws2a[0:128]
    wpack[0:22, O_WS2HI:O_WS2HI + HID] = Ws2a[128:150]
    wpack[0:1, O_WS2B:O_WS2B + HID] = Ws2a[150:151]

    wpack[:, O_WA3:O_WA3 + HID] = np.asarray(Wa3, f32)[:, 0][None, :]
    wpack[:, O_WS3:O_WS3 + HID] = np.asarray(Ws3, f32)[:, 0][None, :]
    wpack[0:9, O_TWID:O_TWID + HID] = (
        np.asarray(width_table, f32) @ Ws1[2 * A + E:]
    )
    wpack[:, O_SCAL] = np.asarray(ba3, f32).reshape(-1)[0]
    wpack[:, O_SCAL + 1] = np.asarray(bs3, f32).reshape(-1)[0]
    wpack[:, O_ID:O_ID + 128] = np.eye(128, dtype=f32)

    in_maps = []
    for c in range(NCORES):
        cols = col_idx[c].reshape(-1)                 # [NPAIR*128]
        sT_t = sTa[:, cols].reshape(KS, 128, NPAIR, 128).transpose(2, 1, 0, 3)
        eT_t = eTa[:, cols].reshape(KE, 128, NPAIR, 128).transpose(2, 1, 0, 3)
        big = np.concatenate(
            [sT_t.reshape(NPAIR, 128, KS * 128),
             eT_t.reshape(NPAIR, 128, KE * 128)], axis=2
        )

        st_c = starts[c * SPC:(c + 1) * SPC].reshape(NPAIR, 2, 128)
        en_c = ends[c * SPC:(c + 1) * SPC].reshape(NPAIR, 2, 128)
        bu_c = bucket[c * SPC:(c + 1) * SPC].reshape(NPAIR, 2, 128)
        t0_c = t0c[c].reshape(NPAIR, 2)

        bmv = np.zeros((NPAIR, 128, 3 * 256), dtype=f32)
        bkt = np.zeros((NPAIR, 9, 256), dtype=f32)
        pp = np.arange(NPAIR)[:, None]
        jj = np.arange(128)[None, :]
        for h in range(2):
            so = st_c[:, h, :] - t0_c[:, h:h + 1]     # [NPAIR,128] in [0,win)
            eo = en_c[:, h, :] - t0_c[:, h:h + 1]
            bmv[pp, so + win * h, 128 * h + jj] = 1.0
            bmv[pp, eo + win * h, 256 + 128 * h + jj] = 1.0
            bkt[pp, bu_c[:, h, :], 128 * h + jj] = 1.0
            cgrid = np.arange(win)[None, :, None]
            m = (cgrid >= so[:, None, :]) & (cgrid <= eo[:, None, :])
            bmv[:, win * h:win * h + win, 512 + 128 * h:512 + 128 * h + 128] = m

        im = {
            "big": np.ascontiguousarray(big),
            "bm": bmv.astype(bf16),
            "bkt": bkt,
            "wpack": wpack,
        }
        in_maps.append(im)
    return in_maps


def kernel(**inputs):
    starts = np.asarray(inputs["span_starts"]).astype(np.int64)
    lengths = np.asarray(inputs["span_lengths"]).astype(np.int64)
    ends = starts + lengths
    spread = ends.reshape(-1, 128).max(axis=1) - starts[0::128] + 1
    assert spread.max() <= WIN, f"window {WIN} too small: {spread.max()}"

    in_maps = _prep_inputs(
        inputs["states"], inputs["embeds"], starts, lengths,
        inputs["Wa1"], inputs["ba1"], inputs["Wa2"], inputs["ba2"],
        inputs["Wa3"], inputs["ba3"], inputs["width_table"],
        inputs["Ws1"], inputs["bs1"], inputs["Ws2"], inputs["bs2"],
        inputs["Ws3"], inputs["bs3"], WIN,
    )
    nc = _build_program(WIN)
    if WIN not in _PROG_CACHE:
        nc.compile()
        _PROG_CACHE[WIN] = nc

    from concourse.bass_utils import run_bass_kernel_spmd
    trace = os.environ.get("KTRACE") == "1"
    try:
        res = run_bass_kernel_spmd(nc, in_maps, core_ids=list(range(NCORES)),
                                   trace=trace)
    except ModuleNotFoundError:
        res = run_bass_kernel_spmd(nc, in_maps, core_ids=list(range(NCORES)))
    if getattr(res, "exec_time_ns", None) is not None:
        print(f"HW exec time: {res.exec_time_ns} ns")
    out = np.concatenate(
        [res.results[c]["scores"].reshape(-1) for c in range(NCORES)]
    )
    return out.astype(np.float32)



# revision 7
# speedup vs baseline: 3.1551x; 3.1551x over previous
"""MentionScore kernel for 8 Trainium2 NeuronCores.

Strategy (data-parallel over spans; all shapes hardcoded):
  T=8192 tokens, A=1024, E=512, HID=150, L=10, S=32768 spans, 8 cores.
  Spans are sorted by start, so core c's 4096 spans touch a contiguous
  ~1055-token range.  Each core receives ONE bf16 DRAM tensor holding
    * a 1280-token slice of [states.T; embeds.T] (12 chunks of 128 feats),
    * 32 span-tile index rows (start/end offset within a statically chosen
      128-token chunk pair, plus width bucket), and
    * all packed weights,
  ~5.6 MB/core total, which keeps the host->device transfer small.

  Algebraic rewrite: with g = [states[st], states[en], pooled, size_emb],
    h1 = g @ Ws1 = states[st] @ W_st + states[en] @ W_en
         + sum_l w[s,l] * (embeds[idx] @ W_em) + width_table[bucket] @ W_wd
  so the big matmuls act on per-token tables computed once per 128-token
  chunk, and the ragged gathers become matmuls with 0/1 selection / softmax
  band matrices that the DEVICE builds from the index rows with fused
  compare ops (no dense band matrices are ever shipped or built on host).

  Because 128 consecutive spans cover <=49 tokens and tile base starts
  deviate <=31 tokens from the uniform 32-per-tile trend, every span tile
  statically fits inside token chunks {j0, j0+1} with
  j0 = clip((32*t - 64)//128, 0, 8); prep asserts this.

All eight cores run the same program on their own 4096-span slice; the only
host-side float math is folding weights/biases (parameter preprocessing).
"""

import numpy as np
import os
import sys

sys.path.insert(0, "/opt/trn_rl_repo")

T, A, E, D, S = 8192, 1024, 512, 20, 32768
HID, L = 150, 10
NCORES = 8
SPC = S // NCORES            # spans per core = 4096
NTILE = SPC // 128           # span tiles per core = 32
NTOK = 1280                  # per-core token-table length
NCH = NTOK // 128            # token chunks = 10
KS = 8                       # feature chunks for states (1024)
KE = 4                       # feature chunks for embeds (512)
BINS = np.array([1, 2, 3, 4, 8, 16, 32, 64], dtype=np.int64)

# column offsets inside the packed per-core tensor [128, COLS] (bf16)
O_TOK = 0                          # (KS+KE) chunks x NTOK token columns
O_IDX = (KS + KE) * NTOK           # [NTILE, 384] block on partitions 0:32
_off = O_IDX + 3 * 128
def _seg(n):
    global _off
    o = _off
    _off += n
    return o
O_WA1 = _seg(KS * HID)
O_WSE = _seg(KS * 2 * HID)
O_WEM = _seg(KE * HID)
O_WA1B = _seg(HID)      # row 0
O_WSEB = _seg(2 * HID)  # row 0
O_WA2LO = _seg(HID)
O_WA2HI = _seg(HID)     # rows 0:22
O_WA2B = _seg(HID)      # row 0
O_WS2LO = _seg(HID)
O_WS2HI = _seg(HID)     # rows 0:22
O_WS2B = _seg(HID)      # row 0
O_WA3 = _seg(HID)       # broadcast to 128 rows
O_WS3 = _seg(HID)
O_TWID = _seg(HID)      # rows 0:9
O_SCAL = _seg(2)        # col0=ba3 col1=bs3 (all rows)
COLS = _off


def _j0(t):
    return min(max((32 * t - 64) // 128, 0), NCH - 2)


_PROG_CACHE = {}


def _build_program():
    if "nc" in _PROG_CACHE:
        return _PROG_CACHE["nc"]
    import concourse.bass as bass
    import concourse.mybir as mybir
    from concourse import tile
    from concourse.bacc import Bacc

    f32 = mybir.dt.float32
    bf16 = mybir.dt.bfloat16
    AF = mybir.ActivationFunctionType
    ALU = mybir.AluOpType
    AX = mybir.AxisListType

    nc = Bacc()

    d_all = nc.dram_tensor("allin", [128, COLS], bf16, kind="ExternalInput")
    d_out = nc.dram_tensor("scores", [128, NTILE], f32, kind="ExternalOutput")

    with tile.TileContext(nc) as tc:
        with (
            tc.tile_pool(name="const", bufs=1) as cpool,
            tc.tile_pool(name="work", bufs=3) as wp,
            tc.tile_pool(name="psT", bufs=2, space=bass.MemorySpace.PSUM) as psT,
            tc.tile_pool(name="psM", bufs=2, space=bass.MemorySpace.PSUM) as psM,
            tc.tile_pool(name="psB", bufs=1, space=bass.MemorySpace.PSUM) as psB,
        ):
            ALLT = cpool.tile([128, COLS], bf16)
            nc.sync.dma_start(ALLT[:], d_all[:])
            # span-tile index rows flattened onto partition 0
            IX = cpool.tile([1, NTILE * 384], bf16)
            nc.sync.dma_start(IX[0:1, :], d_all[0:NTILE, O_IDX:O_IDX + 384])

            pcol = cpool.tile([128, 1], f32)
            nc.gpsimd.iota(pcol[:], [[1, 1]], channel_multiplier=1,
                           allow_small_or_imprecise_dtypes=True)
            crow = cpool.tile([128, 128], f32)
            nc.gpsimd.iota(crow[:], [[1, 128]], channel_multiplier=0,
                           allow_small_or_imprecise_dtypes=True)
            identf = cpool.tile([128, 128], f32)
            nc.vector.tensor_scalar(identf[:], crow[:], pcol[:], None,
                                    ALU.is_equal)
            ones1 = cpool.tile([1, 128], bf16)
            nc.gpsimd.memset(ones1[:], 1.0)
            ones1f = cpool.tile([1, 128], f32)
            nc.gpsimd.memset(ones1f[:], 1.0)
            zbias = cpool.tile([128, 1], f32)
            nc.gpsimd.memset(zbias[:], 0.0)
            # f32 copies of the vector-engine-facing params
            wvec = cpool.tile([128, 302], f32)
            nc.vector.tensor_copy(wvec[:, 0:HID], ALLT[:, O_WA3:O_WA3 + HID])
            nc.vector.tensor_copy(wvec[:, HID:2 * HID],
                                  ALLT[:, O_WS3:O_WS3 + HID])
            nc.vector.tensor_copy(wvec[:, 300:302], ALLT[:, O_SCAL:O_SCAL + 2])
            # f32 copies of layer-2 weights (f32 matmuls need f32 operands)
            WF = cpool.tile([128, 7 * HID], f32)
            for s, off in enumerate((O_WA2LO, O_WA2HI, O_WA2B,
                                     O_WS2LO, O_WS2HI, O_WS2B, O_TWID)):
                nc.vector.tensor_copy(WF[:, HID * s:HID * (s + 1)],
                                      ALLT[:, off:off + HID])
            F_A2LO, F_A2HI, F_A2B = 0, HID, 2 * HID
            F_S2LO, F_S2HI, F_S2B, F_TWID = (3 * HID, 4 * HID, 5 * HID,
                                             6 * HID)

            TBL = cpool.tile([128, NCH * 451], f32)   # [tse(300)|temb(150)|1]
            EWT = cpool.tile([128, NCH], f32)         # exp(attn) per token
            OUT = cpool.tile([128, NTILE], f32)

            # ---- per-token tables, one 128-token chunk at a time ----
            for j in range(NCH):
                tbp = psT.tile([128, 451], f32, tag="tbl")
                for k in range(KS):
                    nc.tensor.matmul(
                        tbp[:, 0:300],
                        ALLT[:, k * NTOK + 128 * j:k * NTOK + 128 * j + 128],
                        ALLT[:, O_WSE + 300 * k:O_WSE + 300 * (k + 1)],
                        start=(k == 0), stop=False)
                nc.tensor.matmul(tbp[:, 0:300], ones1[:],
                                 ALLT[0:1, O_WSEB:O_WSEB + 300],
                                 start=False, stop=True)
                for k in range(KE):
                    nc.tensor.matmul(
                        tbp[:, 300:450],
                        ALLT[:, (KS + k) * NTOK + 128 * j:
                             (KS + k) * NTOK + 128 * j + 128],
                        ALLT[:, O_WEM + HID * k:O_WEM + HID * (k + 1)],
                        start=(k == 0), stop=(k == KE - 1))

                a1p = psM.tile([128, HID], f32, tag="mm150")
                for k in range(KS):
                    nc.tensor.matmul(
                        a1p[:],
                        ALLT[:, k * NTOK + 128 * j:k * NTOK + 128 * j + 128],
                        ALLT[:, O_WA1 + HID * k:O_WA1 + HID * (k + 1)],
                        start=(k == 0), stop=False)
                nc.tensor.matmul(a1p[:], ones1[:],
                                 ALLT[0:1, O_WA1B:O_WA1B + HID],
                                 start=False, stop=True)
                a1r = wp.tile([128, HID], f32, tag="a1r")
                nc.scalar.activation(a1r[:], a1p[:], AF.Relu, bias=zbias[:])

                tpc = psB.tile([128, 256], f32, tag="tpAB")
                nc.tensor.transpose(tpc[:, 0:128], a1r[:, 0:128], identf[:])
                nc.tensor.transpose(tpc[0:22, 128:256], a1r[:, 128:HID],
                                    identf[:])
                a1Tlo = wp.tile([128, 128], f32, tag="aTlo")
                nc.vector.tensor_copy(a1Tlo[:], tpc[:, 0:128])
                a1Thi = wp.tile([22, 128], f32, tag="aThi")
                nc.vector.tensor_copy(a1Thi[:], tpc[0:22, 128:256])

                a2p = psM.tile([128, HID], f32, tag="mm150")
                nc.tensor.matmul(a2p[:], a1Tlo[:],
                                 WF[:, F_A2LO:F_A2LO + HID],
                                 start=True, stop=False)
                nc.tensor.matmul(a2p[:], a1Thi[:],
                                 WF[0:22, F_A2HI:F_A2HI + HID],
                                 start=False, stop=False)
                nc.tensor.matmul(a2p[:], ones1f[:],
                                 WF[0:1, F_A2B:F_A2B + HID],
                                 start=False, stop=True)
                a2r = wp.tile([128, HID], f32, tag="a2r")
                nc.scalar.activation(a2r[:], a2p[:], AF.Relu, bias=zbias[:])

                atmp = wp.tile([128, HID], f32, tag="atmp")
                nc.vector.tensor_tensor(atmp[:], a2r[:], wvec[:, 0:HID],
                                        op=ALU.mult)
                adot = wp.tile([128, 1], f32, tag="adot")
                nc.vector.tensor_reduce(adot[:], atmp[:], axis=AX.X,
                                        op=ALU.add)
                nc.scalar.activation(EWT[:, j:j + 1], adot[:], AF.Exp,
                                     bias=wvec[:, 300:301])

                nc.vector.tensor_copy(TBL[:, 451 * j:451 * j + 450],
                                      tbp[:, 0:450])
                nc.vector.memset(TBL[:, 451 * j + 450:451 * j + 451], 1.0)

            # ---- per span tile: build band matrices, contract, score ----
            for t in range(NTILE):
                j0 = _j0(t)
                bc = psB.tile([128, 384], f32, tag="bc")
                nc.tensor.matmul(bc[:], ones1[:],
                                 IX[0:1, 384 * t:384 * (t + 1)],
                                 start=True, stop=True)

                sels = []
                bws = []
                for i in range(2):
                    ds = wp.tile([128, 256], f32, tag=f"ds{i}")
                    if i == 0:
                        nc.vector.tensor_scalar(ds[:], bc[:, 0:256], pcol[:],
                                                None, ALU.subtract)
                    else:
                        nc.vector.tensor_scalar(ds[:], bc[:, 0:256], pcol[:],
                                                128.0, ALU.subtract,
                                                ALU.subtract)
                    sel = wp.tile([128, 256], f32, tag=f"sel{i}")
                    nc.vector.tensor_scalar(sel[:], ds[:], 0.0, None,
                                            ALU.is_equal)
                    ge = wp.tile([128, 128], f32, tag="ge")
                    nc.vector.tensor_scalar(ge[:], ds[:, 0:128], 0.0, None,
                                            ALU.is_le)
                    le = wp.tile([128, 128], f32, tag="le")
                    nc.vector.tensor_scalar(le[:], ds[:, 128:256], 0.0, None,
                                            ALU.is_ge)
                    mask = wp.tile([128, 128], f32, tag="mask")
                    nc.vector.tensor_tensor(mask[:], ge[:], le[:], op=ALU.mult)
                    bw = wp.tile([128, 128], f32, tag=f"bw{i}")
                    nc.vector.tensor_scalar_mul(bw[:], mask[:],
                                                EWT[:, j0 + i:j0 + i + 1])
                    sels.append(sel)
                    bws.append(bw)
                db = wp.tile([128, 128], f32, tag="db")
                nc.vector.tensor_scalar(db[:], bc[:, 256:384], pcol[:], 0.0,
                                        ALU.subtract, ALU.is_equal)

                hAB = psB.tile([128, 2 * HID + 1], f32, tag="hAB")
                hA = hAB[:, 0:HID]
                hB = hAB[:, HID:2 * HID + 1]
                for i in range(2):
                    o = 451 * (j0 + i)
                    nc.tensor.matmul(hA, sels[i][:, 0:128],
                                     TBL[:, o:o + HID],
                                     start=(i == 0), stop=False)
                    nc.tensor.matmul(hA, sels[i][:, 128:256],
                                     TBL[:, o + HID:o + 2 * HID],
                                     start=False, stop=False)
                nc.tensor.matmul(hA, db[0:9, :],
                                 WF[0:9, F_TWID:F_TWID + HID],
                                 start=False, stop=True)
                for i in range(2):
                    o = 451 * (j0 + i)
                    nc.tensor.matmul(hB, bws[i][:],
                                     TBL[:, o + 300:o + 451],
                                     start=(i == 0), stop=(i == 1))

                rec = wp.tile([128, 1], f32, tag="rec")
                nc.vector.reciprocal(rec[:], hB[:, HID:HID + 1])
                hBs = wp.tile([128, HID], f32, tag="hBs")
                nc.vector.tensor_scalar_mul(hBs[:], hB[:, 0:HID], rec[:])
                h1s = wp.tile([128, HID], f32, tag="h1s")
                nc.vector.tensor_tensor(h1s[:], hA, hBs[:], op=ALU.add)
                h1r = wp.tile([128, HID], f32, tag="h1r")
                nc.scalar.activation(h1r[:], h1s[:], AF.Relu, bias=zbias[:])

                tqc = psB.tile([128, 256], f32, tag="tpAB")
                nc.tensor.transpose(tqc[:, 0:128], h1r[:, 0:128], identf[:])
                nc.tensor.transpose(tqc[0:22, 128:256], h1r[:, 128:HID],
                                    identf[:])
                h1Tlo = wp.tile([128, 128], f32, tag="aTlo")
                nc.vector.tensor_copy(h1Tlo[:], tqc[:, 0:128])
                h1Thi = wp.tile([22, 128], f32, tag="aThi")
                nc.vector.tensor_copy(h1Thi[:], tqc[0:22, 128:256])

                h2p = psM.tile([128, HID], f32, tag="mm150")
                nc.tensor.matmul(h2p[:], h1Tlo[:],
                                 WF[:, F_S2LO:F_S2LO + HID],
                                 start=True, stop=False)
                nc.tensor.matmul(h2p[:], h1Thi[:],
                                 WF[0:22, F_S2HI:F_S2HI + HID],
                                 start=False, stop=False)
                nc.tensor.matmul(h2p[:], ones1f[:],
                                 WF[0:1, F_S2B:F_S2B + HID],
                                 start=False, stop=True)
                h2r = wp.tile([128, HID], f32, tag="h2r")
                nc.scalar.activation(h2r[:], h2p[:], AF.Relu, bias=zbias[:])

                stmp = wp.tile([128, HID], f32, tag="stmp")
                nc.vector.tensor_tensor(stmp[:], h2r[:],
                                        wvec[:, HID:2 * HID], op=ALU.mult)
                sdot = wp.tile([128, 1], f32, tag="sdot")
                nc.vector.tensor_reduce(sdot[:], stmp[:], axis=AX.X,
                                        op=ALU.add)
                nc.vector.tensor_scalar(OUT[:, t:t + 1], sdot[:],
                                        wvec[:, 301:302], None, ALU.add)

            nc.sync.dma_start(d_out[:], OUT[:])

    return nc


def _prep_inputs(states, embeds, starts, lengths,
                 Wa1, ba1, Wa2, ba2, Wa3, ba3,
                 width_table, Ws1, bs1, Ws2, bs2, Ws3, bs3):
    import ml_dtypes
    bf16 = ml_dtypes.bfloat16
    f32 = np.float32

    ends = starts + lengths
    bucket = np.searchsorted(BINS, lengths + 1, side="right")

    # feature-major token matrix [1536, T] in bf16
    Fb = np.empty(((KS + KE) * 128, T), dtype=bf16)
    Fb[0:A] = np.asarray(states, f32).T
    Fb[A:] = np.asarray(embeds, f32).T

    # ---- packed weights (identical across cores) ----
    Ws1 = np.asarray(Ws1, f32)
    wpk = np.zeros((128, COLS - O_WA1), dtype=f32)
    def put(off, arr, r0=0):
        a = np.asarray(arr, f32)
        wpk[r0:r0 + a.shape[0], off - O_WA1:off - O_WA1 + a.shape[1]] = a
    Wa1 = np.asarray(Wa1, f32)
    for k in range(KS):
        put(O_WA1 + HID * k, Wa1[128 * k:128 * (k + 1)])
        put(O_WSE + 2 * HID * k,
            np.hstack([Ws1[128 * k:128 * (k + 1)],
                       Ws1[A + 128 * k:A + 128 * (k + 1)]]))
    for k in range(KE):
        put(O_WEM + HID * k, Ws1[2 * A + 128 * k:2 * A + 128 * (k + 1)])
    put(O_WA1B, np.asarray(ba1, f32)[None, :])
    put(O_WSEB, np.hstack([np.asarray(bs1, f32), np.zeros(HID, f32)])[None, :])
    Wa2 = np.asarray(Wa2, f32)
    put(O_WA2LO, Wa2[0:128])
    put(O_WA2HI, Wa2[128:HID])
    put(O_WA2B, np.asarray(ba2, f32)[None, :])
    Ws2 = np.asarray(Ws2, f32)
    put(O_WS2LO, Ws2[0:128])
    put(O_WS2HI, Ws2[128:HID])
    put(O_WS2B, np.asarray(bs2, f32)[None, :])
    wpk[:, O_WA3 - O_WA1:O_WA3 - O_WA1 + HID] = np.asarray(Wa3, f32)[:, 0]
    wpk[:, O_WS3 - O_WA1:O_WS3 - O_WA1 + HID] = np.asarray(Ws3, f32)[:, 0]
    put(O_TWID, np.asarray(width_table, f32) @ Ws1[2 * A + E:])
    wpk[:, O_SCAL - O_WA1] = np.asarray(ba3, f32).reshape(-1)[0]
    wpk[:, O_SCAL - O_WA1 + 1] = np.asarray(bs3, f32).reshape(-1)[0]
    wpb = wpk.astype(bf16)

    j0s = np.array([_j0(t) for t in range(NTILE)], dtype=np.int64)

    in_maps = []
    for c in range(NCORES):
        allc = np.zeros((128, COLS), dtype=bf16)
        lo = max(int(starts[c * SPC]) - 8, 0)
        hi = min(lo + NTOK, T)
        n = hi - lo
        tokv = allc[:, O_TOK:O_TOK + (KS + KE) * NTOK]
        tokv = tokv.reshape(128, KS + KE, NTOK)
        tokv[:, :, 0:n] = Fb[:, lo:hi].reshape(KS + KE, 128, n).transpose(1, 0, 2)

        st_c = starts[c * SPC:(c + 1) * SPC].reshape(NTILE, 128) - lo
        en_c = ends[c * SPC:(c + 1) * SPC].reshape(NTILE, 128) - lo
        assert en_c.max() < n, "token table too small"
        st_r = st_c - 128 * j0s[:, None]
        en_r = en_c - 128 * j0s[:, None]
        assert st_r.min() >= 0 and en_r.max() < 256, \
            "static chunk rule violated"
        bu_c = bucket[c * SPC:(c + 1) * SPC].reshape(NTILE, 128)
        idx = np.concatenate([st_r, en_r, bu_c], axis=1).astype(f32)
        allc[0:NTILE, O_IDX:O_IDX + 384] = idx
        allc[:, O_WA1:] = wpb
        in_maps.append({"allin": allc})
    return in_maps


def _enable_jax_cache():
    try:
        import jax
        jax.config.update("jax_compilation_cache_dir", "/tmp/.jax_nc_cache")
        jax.config.update("jax_persistent_cache_min_compile_time_secs", 0)
        jax.config.update("jax_persistent_cache_min_entry_size_bytes", -1)
    except Exception:
        pass


def kernel(**inputs):
    _enable_jax_cache()
    starts = np.asarray(inputs["span_starts"]).astype(np.int64)
    lengths = np.asarray(inputs["span_lengths"]).astype(np.int64)

    in_maps = _prep_inputs(
        inputs["states"], inputs["embeds"], starts, lengths,
        inputs["Wa1"], inputs["ba1"], inputs["Wa2"], inputs["ba2"],
        inputs["Wa3"], inputs["ba3"], inputs["width_table"],
        inputs["Ws1"], inputs["bs1"], inputs["Ws2"], inputs["bs2"],
        inputs["Ws3"], inputs["bs3"],
    )
    nc = _build_program()
    if "nc" not in _PROG_CACHE:
        nc.compile()
        _PROG_CACHE["nc"] = nc

    from concourse.bass_utils import run_bass_kernel_spmd
    res = run_bass_kernel_spmd(nc, in_maps, core_ids=list(range(NCORES)))
    if getattr(res, "exec_time_ns", None) is not None:
        print(f"HW exec time: {res.exec_time_ns} ns")
    out = np.concatenate(
        [res.results[c]["scores"].T.reshape(-1) for c in range(NCORES)]
    )
    return out.astype(np.float32)


# revision 10
# speedup vs baseline: 4.0074x; 1.2701x over previous
"""MentionScore kernel for 8 Trainium2 NeuronCores.

Strategy (data-parallel over spans; all shapes hardcoded):
  T=8192 tokens, A=1024, E=512, HID=150, L=10, S=32768 spans, 8 cores.
  Spans are sorted by start, so core c's 4096 spans touch a contiguous
  ~1055-token range.  Each core receives ONE bf16 DRAM tensor holding
    * a 1280-token slice of [states.T; embeds.T] (12 chunks of 128 feats),
    * 32 span-tile index rows (start/end offset within a statically chosen
      128-token chunk pair, plus width bucket), and
    * all packed weights,
  ~5.6 MB/core total, which keeps the host->device transfer small.

  Algebraic rewrite: with g = [states[st], states[en], pooled, size_emb],
    h1 = g @ Ws1 = states[st] @ W_st + states[en] @ W_en
         + sum_l w[s,l] * (embeds[idx] @ W_em) + width_table[bucket] @ W_wd
  so the big matmuls act on per-token tables computed once per 128-token
  chunk, and the ragged gathers become matmuls with 0/1 selection / softmax
  band matrices that the DEVICE builds from the index rows with fused
  compare ops (no dense band matrices are ever shipped or built on host).

  Because 128 consecutive spans cover <=49 tokens and tile base starts
  deviate <=31 tokens from the uniform 32-per-tile trend, every span tile
  statically fits inside token chunks {j0, j0+1} with
  j0 = clip((32*t - 64)//128, 0, 8); prep asserts this.

All eight cores run the same program on their own 4096-span slice; the only
host-side float math is folding weights/biases (parameter preprocessing).
"""

import numpy as np
import os
import sys

sys.path.insert(0, "/opt/trn_rl_repo")

T, A, E, D, S = 8192, 1024, 512, 20, 32768
HID, L = 150, 10
NCORES = 8
SPC = S // NCORES            # spans per core = 4096
NTILE = SPC // 128           # span tiles per core = 32
NTOK = 1152                  # per-core token-table length
NCH = NTOK // 128            # token chunks = 9
KS = 8                       # feature chunks for states (1024)
KE = 4                       # feature chunks for embeds (512)
BINS = np.array([1, 2, 3, 4, 8, 16, 32, 64], dtype=np.int64)

# column offsets inside the packed per-core tensor [128, COLS] (bf16)
O_TOK = 0                          # (KS+KE) chunks x NTOK token columns
O_IDX = (KS + KE) * NTOK           # [NTILE, 384] block on partitions 0:32
_off = O_IDX + 3 * 128
def _seg(n):
    global _off
    o = _off
    _off += n
    return o
O_WA1 = _seg(KS * HID)
O_WSE = _seg(KS * 2 * HID)
O_WEM = _seg(KE * HID)
O_WA1B = _seg(HID)      # row 0
O_WSEB = _seg(2 * HID)  # row 0
O_WA2LO = _seg(HID)
O_WA2HI = _seg(HID)     # rows 0:22
O_WA2B = _seg(HID)      # row 0
O_WS2LO = _seg(HID)
O_WS2HI = _seg(HID)     # rows 0:22
O_WS2B = _seg(HID)      # row 0
O_WA3 = _seg(HID)       # broadcast to 128 rows
O_WS3 = _seg(HID)
O_TWID = _seg(HID)      # rows 0:9
O_SCAL = _seg(2)        # col0=ba3 col1=bs3 (all rows)
COLS = _off


def _j0(t):
    return min(max((32 * t - 64) // 128, 0), NCH - 2)


_PROG_CACHE = {}


def _build_program():
    if "nc" in _PROG_CACHE:
        return _PROG_CACHE["nc"]
    import concourse.bass as bass
    import concourse.mybir as mybir
    from concourse import tile
    from concourse.bacc import Bacc

    f32 = mybir.dt.float32
    bf16 = mybir.dt.bfloat16
    AF = mybir.ActivationFunctionType
    ALU = mybir.AluOpType
    AX = mybir.AxisListType

    nc = Bacc()

    d_all = nc.dram_tensor("allin", [128, COLS], bf16, kind="ExternalInput")
    d_out = nc.dram_tensor("scores", [128, NTILE], f32, kind="ExternalOutput")

    with tile.TileContext(nc) as tc:
        with (
            tc.tile_pool(name="const", bufs=1) as cpool,
            tc.tile_pool(name="work", bufs=3) as wp,
            tc.tile_pool(name="psT", bufs=2, space=bass.MemorySpace.PSUM) as psT,
            tc.tile_pool(name="psM", bufs=2, space=bass.MemorySpace.PSUM) as psM,
            tc.tile_pool(name="psB", bufs=1, space=bass.MemorySpace.PSUM) as psB,
        ):
            ALLT = cpool.tile([128, COLS], bf16)
            nc.sync.dma_start(ALLT[:], d_all[:])
            # span-tile index rows flattened onto partition 0
            IX = cpool.tile([1, NTILE * 384], bf16)
            nc.sync.dma_start(IX[0:1, :], d_all[0:NTILE, O_IDX:O_IDX + 384])

            pcol = cpool.tile([128, 1], f32)
            nc.gpsimd.iota(pcol[:], [[1, 1]], channel_multiplier=1,
                           allow_small_or_imprecise_dtypes=True)
            crow = cpool.tile([128, 128], f32)
            nc.gpsimd.iota(crow[:], [[1, 128]], channel_multiplier=0,
                           allow_small_or_imprecise_dtypes=True)
            identf = cpool.tile([128, 128], f32)
            nc.vector.tensor_scalar(identf[:], crow[:], pcol[:], None,
                                    ALU.is_equal)
            ones1 = cpool.tile([1, 128], bf16)
            nc.gpsimd.memset(ones1[:], 1.0)
            ones1f = cpool.tile([1, 128], f32)
            nc.gpsimd.memset(ones1f[:], 1.0)
            zbias = cpool.tile([128, 1], f32)
            nc.gpsimd.memset(zbias[:], 0.0)
            # f32 copies of the vector-engine-facing params
            wvec = cpool.tile([128, 302], f32)
            nc.vector.tensor_copy(wvec[:, 0:HID], ALLT[:, O_WA3:O_WA3 + HID])
            nc.vector.tensor_copy(wvec[:, HID:2 * HID],
                                  ALLT[:, O_WS3:O_WS3 + HID])
            nc.vector.tensor_copy(wvec[:, 300:302], ALLT[:, O_SCAL:O_SCAL + 2])
            # f32 copies of layer-2 weights (f32 matmuls need f32 operands)
            WF = cpool.tile([128, 7 * HID], f32)
            for s, off in enumerate((O_WA2LO, O_WA2HI, O_WA2B,
                                     O_WS2LO, O_WS2HI, O_WS2B, O_TWID)):
                nc.vector.tensor_copy(WF[:, HID * s:HID * (s + 1)],
                                      ALLT[:, off:off + HID])
            F_A2LO, F_A2HI, F_A2B = 0, HID, 2 * HID
            F_S2LO, F_S2HI, F_S2B, F_TWID = (3 * HID, 4 * HID, 5 * HID,
                                             6 * HID)

            TBL = cpool.tile([128, NCH * 451], f32)   # [tse(300)|temb(150)|1]
            EWT = cpool.tile([128, NCH], f32)         # exp(attn) per token
            OUT = cpool.tile([128, NTILE], f32)

            # ---- per-token tables, one 128-token chunk at a time ----
            for j in range(NCH):
                tbp = psT.tile([128, 451], f32, tag="tbl")
                for k in range(KS):
                    nc.tensor.matmul(
                        tbp[:, 0:300],
                        ALLT[:, k * NTOK + 128 * j:k * NTOK + 128 * j + 128],
                        ALLT[:, O_WSE + 300 * k:O_WSE + 300 * (k + 1)],
                        start=(k == 0), stop=False)
                nc.tensor.matmul(tbp[:, 0:300], ones1[:],
                                 ALLT[0:1, O_WSEB:O_WSEB + 300],
                                 start=False, stop=True)
                for k in range(KE):
                    nc.tensor.matmul(
                        tbp[:, 300:450],
                        ALLT[:, (KS + k) * NTOK + 128 * j:
                             (KS + k) * NTOK + 128 * j + 128],
                        ALLT[:, O_WEM + HID * k:O_WEM + HID * (k + 1)],
                        start=(k == 0), stop=(k == KE - 1))

                a1p = psM.tile([128, HID], f32, tag="mm150")
                for k in range(KS):
                    nc.tensor.matmul(
                        a1p[:],
                        ALLT[:, k * NTOK + 128 * j:k * NTOK + 128 * j + 128],
                        ALLT[:, O_WA1 + HID * k:O_WA1 + HID * (k + 1)],
                        start=(k == 0), stop=False)
                nc.tensor.matmul(a1p[:], ones1[:],
                                 ALLT[0:1, O_WA1B:O_WA1B + HID],
                                 start=False, stop=True)
                a1r = wp.tile([128, HID], f32, tag="a1r")
                nc.scalar.activation(a1r[:], a1p[:], AF.Relu, bias=zbias[:])

                tpc = psB.tile([128, 256], f32, tag="tpAB")
                nc.tensor.transpose(tpc[:, 0:128], a1r[:, 0:128], identf[:])
                nc.tensor.transpose(tpc[0:22, 128:256], a1r[:, 128:HID],
                                    identf[:])
                a1Tlo = wp.tile([128, 128], f32, tag="aTlo")
                nc.vector.tensor_copy(a1Tlo[:], tpc[:, 0:128])
                a1Thi = wp.tile([22, 128], f32, tag="aThi")
                nc.vector.tensor_copy(a1Thi[:], tpc[0:22, 128:256])

                a2p = psM.tile([128, HID], f32, tag="mm150")
                nc.tensor.matmul(a2p[:], a1Tlo[:],
                                 WF[:, F_A2LO:F_A2LO + HID],
                                 start=True, stop=False)
                nc.tensor.matmul(a2p[:], a1Thi[:],
                                 WF[0:22, F_A2HI:F_A2HI + HID],
                                 start=False, stop=False)
                nc.tensor.matmul(a2p[:], ones1f[:],
                                 WF[0:1, F_A2B:F_A2B + HID],
                                 start=False, stop=True)
                a2r = wp.tile([128, HID], f32, tag="a2r")
                nc.scalar.activation(a2r[:], a2p[:], AF.Relu, bias=zbias[:])

                atmp = wp.tile([128, HID], f32, tag="atmp")
                nc.vector.tensor_tensor(atmp[:], a2r[:], wvec[:, 0:HID],
                                        op=ALU.mult)
                adot = wp.tile([128, 1], f32, tag="adot")
                nc.vector.tensor_reduce(adot[:], atmp[:], axis=AX.X,
                                        op=ALU.add)
                nc.scalar.activation(EWT[:, j:j + 1], adot[:], AF.Exp,
                                     bias=wvec[:, 300:301])

                nc.vector.tensor_copy(TBL[:, 451 * j:451 * j + 450],
                                      tbp[:, 0:450])
                nc.vector.memset(TBL[:, 451 * j + 450:451 * j + 451], 1.0)

            # ---- per span tile: build band matrices, contract, score ----
            for t in range(NTILE):
                j0 = _j0(t)
                bc = psB.tile([128, 384], f32, tag="bc")
                nc.tensor.matmul(bc[:], ones1[:],
                                 IX[0:1, 384 * t:384 * (t + 1)],
                                 start=True, stop=True)

                sels = []
                bws = []
                for i in range(2):
                    ds = wp.tile([128, 256], f32, tag=f"ds{i}")
                    if i == 0:
                        nc.vector.tensor_scalar(ds[:], bc[:, 0:256], pcol[:],
                                                None, ALU.subtract)
                    else:
                        nc.vector.tensor_scalar(ds[:], bc[:, 0:256], pcol[:],
                                                128.0, ALU.subtract,
                                                ALU.subtract)
                    sel = wp.tile([128, 256], f32, tag=f"sel{i}")
                    nc.vector.tensor_scalar(sel[:], ds[:], 0.0, None,
                                            ALU.is_equal)
                    ge = wp.tile([128, 128], f32, tag="ge")
                    nc.vector.tensor_scalar(ge[:], ds[:, 0:128], 0.0, None,
                                            ALU.is_le)
                    le = wp.tile([128, 128], f32, tag="le")
                    nc.vector.tensor_scalar(le[:], ds[:, 128:256], 0.0, None,
                                            ALU.is_ge)
                    mask = wp.tile([128, 128], f32, tag="mask")
                    nc.vector.tensor_tensor(mask[:], ge[:], le[:], op=ALU.mult)
                    bw = wp.tile([128, 128], f32, tag=f"bw{i}")
                    nc.vector.tensor_scalar_mul(bw[:], mask[:],
                                                EWT[:, j0 + i:j0 + i + 1])
                    sels.append(sel)
                    bws.append(bw)
                db = wp.tile([128, 128], f32, tag="db")
                nc.vector.tensor_scalar(db[:], bc[:, 256:384], pcol[:], 0.0,
                                        ALU.subtract, ALU.is_equal)

                hAB = psB.tile([128, 2 * HID + 1], f32, tag="hAB")
                hA = hAB[:, 0:HID]
                hB = hAB[:, HID:2 * HID + 1]
                for i in range(2):
                    o = 451 * (j0 + i)
                    nc.tensor.matmul(hA, sels[i][:, 0:128],
                                     TBL[:, o:o + HID],
                                     start=(i == 0), stop=False)
                    nc.tensor.matmul(hA, sels[i][:, 128:256],
                                     TBL[:, o + HID:o + 2 * HID],
                                     start=False, stop=False)
                nc.tensor.matmul(hA, db[0:9, :],
                                 WF[0:9, F_TWID:F_TWID + HID],
                                 start=False, stop=True)
                for i in range(2):
                    o = 451 * (j0 + i)
                    nc.tensor.matmul(hB, bws[i][:],
                                     TBL[:, o + 300:o + 451],
                                     start=(i == 0), stop=(i == 1))

                rec = wp.tile([128, 1], f32, tag="rec")
                nc.vector.reciprocal(rec[:], hB[:, HID:HID + 1])
                hBs = wp.tile([128, HID], f32, tag="hBs")
                nc.vector.tensor_scalar_mul(hBs[:], hB[:, 0:HID], rec[:])
                h1s = wp.tile([128, HID], f32, tag="h1s")
                nc.vector.tensor_tensor(h1s[:], hA, hBs[:], op=ALU.add)
                h1r = wp.tile([128, HID], f32, tag="h1r")
                nc.scalar.activation(h1r[:], h1s[:], AF.Relu, bias=zbias[:])

                tqc = psB.tile([128, 256], f32, tag="tpAB")
                nc.tensor.transpose(tqc[:, 0:128], h1r[:, 0:128], identf[:])
                nc.tensor.transpose(tqc[0:22, 128:256], h1r[:, 128:HID],
                                    identf[:])
                h1Tlo = wp.tile([128, 128], f32, tag="aTlo")
                nc.vector.tensor_copy(h1Tlo[:], tqc[:, 0:128])
                h1Thi = wp.tile([22, 128], f32, tag="aThi")
                nc.vector.tensor_copy(h1Thi[:], tqc[0:22, 128:256])

                h2p = psM.tile([128, HID], f32, tag="mm150")
                nc.tensor.matmul(h2p[:], h1Tlo[:],
                                 WF[:, F_S2LO:F_S2LO + HID],
                                 start=True, stop=False)
                nc.tensor.matmul(h2p[:], h1Thi[:],
                                 WF[0:22, F_S2HI:F_S2HI + HID],
                                 start=False, stop=False)
                nc.tensor.matmul(h2p[:], ones1f[:],
                                 WF[0:1, F_S2B:F_S2B + HID],
                                 start=False, stop=True)
                h2r = wp.tile([128, HID], f32, tag="h2r")
                nc.scalar.activation(h2r[:], h2p[:], AF.Relu, bias=zbias[:])

                stmp = wp.tile([128, HID], f32, tag="stmp")
                nc.vector.tensor_tensor(stmp[:], h2r[:],
                                        wvec[:, HID:2 * HID], op=ALU.mult)
                sdot = wp.tile([128, 1], f32, tag="sdot")
                nc.vector.tensor_reduce(sdot[:], stmp[:], axis=AX.X,
                                        op=ALU.add)
                nc.vector.tensor_scalar(OUT[:, t:t + 1], sdot[:],
                                        wvec[:, 301:302], None, ALU.add)

            nc.sync.dma_start(d_out[:], OUT[:])

    return nc


def _prep_inputs(states, embeds, starts, lengths,
                 Wa1, ba1, Wa2, ba2, Wa3, ba3,
                 width_table, Ws1, bs1, Ws2, bs2, Ws3, bs3):
    import ml_dtypes
    bf16 = ml_dtypes.bfloat16
    f32 = np.float32

    ends = starts + lengths
    bucket = np.searchsorted(BINS, lengths + 1, side="right")

    # token features as [partition, chunk, token] strided views in bf16
    sT = np.asarray(states, f32).T.astype(bf16)    # [1024, T]
    eT = np.asarray(embeds, f32).T.astype(bf16)    # [512, T]
    sv = sT.reshape(KS, 128, T).transpose(1, 0, 2)  # [128, KS, T] view
    ev = eT.reshape(KE, 128, T).transpose(1, 0, 2)  # [128, KE, T] view

    # ---- packed weights (identical across cores) ----
    Ws1 = np.asarray(Ws1, f32)
    wpk = np.zeros((128, COLS - O_WA1), dtype=f32)
    def put(off, arr, r0=0):
        a = np.asarray(arr, f32)
        wpk[r0:r0 + a.shape[0], off - O_WA1:off - O_WA1 + a.shape[1]] = a
    Wa1 = np.asarray(Wa1, f32)
    for k in range(KS):
        put(O_WA1 + HID * k, Wa1[128 * k:128 * (k + 1)])
        put(O_WSE + 2 * HID * k,
            np.hstack([Ws1[128 * k:128 * (k + 1)],
                       Ws1[A + 128 * k:A + 128 * (k + 1)]]))
    for k in range(KE):
        put(O_WEM + HID * k, Ws1[2 * A + 128 * k:2 * A + 128 * (k + 1)])
    put(O_WA1B, np.asarray(ba1, f32)[None, :])
    put(O_WSEB, np.hstack([np.asarray(bs1, f32), np.zeros(HID, f32)])[None, :])
    Wa2 = np.asarray(Wa2, f32)
    put(O_WA2LO, Wa2[0:128])
    put(O_WA2HI, Wa2[128:HID])
    put(O_WA2B, np.asarray(ba2, f32)[None, :])
    Ws2 = np.asarray(Ws2, f32)
    put(O_WS2LO, Ws2[0:128])
    put(O_WS2HI, Ws2[128:HID])
    put(O_WS2B, np.asarray(bs2, f32)[None, :])
    wpk[:, O_WA3 - O_WA1:O_WA3 - O_WA1 + HID] = np.asarray(Wa3, f32)[:, 0]
    wpk[:, O_WS3 - O_WA1:O_WS3 - O_WA1 + HID] = np.asarray(Ws3, f32)[:, 0]
    put(O_TWID, np.asarray(width_table, f32) @ Ws1[2 * A + E:])
    wpk[:, O_SCAL - O_WA1] = np.asarray(ba3, f32).reshape(-1)[0]
    wpk[:, O_SCAL - O_WA1 + 1] = np.asarray(bs3, f32).reshape(-1)[0]
    wpb = wpk.astype(bf16)

    j0s = np.array([_j0(t) for t in range(NTILE)], dtype=np.int64)

    in_maps = []
    for c in range(NCORES):
        allc = np.zeros((128, COLS), dtype=bf16)
        lo = max(int(starts[c * SPC]) - 8, 0)
        hi = min(lo + NTOK, T)
        n = hi - lo
        tokv = allc[:, O_TOK:O_TOK + (KS + KE) * NTOK]
        tokv = tokv.reshape(128, KS + KE, NTOK)
        tokv[:, 0:KS, 0:n] = sv[:, :, lo:hi]
        tokv[:, KS:, 0:n] = ev[:, :, lo:hi]

        st_c = starts[c * SPC:(c + 1) * SPC].reshape(NTILE, 128) - lo
        en_c = ends[c * SPC:(c + 1) * SPC].reshape(NTILE, 128) - lo
        assert en_c.max() < n, "token table too small"
        st_r = st_c - 128 * j0s[:, None]
        en_r = en_c - 128 * j0s[:, None]
        assert st_r.min() >= 0 and en_r.max() < 256, \
            "static chunk rule violated"
        bu_c = bucket[c * SPC:(c + 1) * SPC].reshape(NTILE, 128)
        idx = np.concatenate([st_r, en_r, bu_c], axis=1).astype(f32)
        allc[0:NTILE, O_IDX:O_IDX + 384] = idx
        allc[:, O_WA1:] = wpb
        in_maps.append({"allin": allc})
    return in_maps


def _enable_jax_cache():
    try:
        import jax
        jax.config.update("jax_compilation_cache_dir", "/tmp/.jax_nc_cache")
        jax.config.update("jax_persistent_cache_min_compile_time_secs", 0)
        jax.config.update("jax_persistent_cache_min_entry_size_bytes", -1)
    except Exception:
        pass


def kernel(**inputs):
    _enable_jax_cache()
    starts = np.asarray(inputs["span_starts"]).astype(np.int64)
    lengths = np.asarray(inputs["span_lengths"]).astype(np.int64)

    in_maps = _prep_inputs(
        inputs["states"], inputs["embeds"], starts, lengths,
        inputs["Wa1"], inputs["ba1"], inputs["Wa2"], inputs["ba2"],
        inputs["Wa3"], inputs["ba3"], inputs["width_table"],
        inputs["Ws1"], inputs["bs1"], inputs["Ws2"], inputs["bs2"],
        inputs["Ws3"], inputs["bs3"],
    )
    nc = _build_program()
    if "nc" not in _PROG_CACHE:
        nc.compile()
        _PROG_CACHE["nc"] = nc

    from concourse.bass_utils import run_bass_kernel_spmd
    res = run_bass_kernel_spmd(nc, in_maps, core_ids=list(range(NCORES)))
    if getattr(res, "exec_time_ns", None) is not None:
        print(f"HW exec time: {res.exec_time_ns} ns")
    out = np.concatenate(
        [res.results[c]["scores"].T.reshape(-1) for c in range(NCORES)]
    )
    return out.astype(np.float32)
